# revision 18
# baseline (speedup 1.0000x reference)
"""BinaryLinear (sign-binarized weight linear layer) on 8 Trainium2 NeuronCores.

y[b,s,o] = sum_i x[b,s,i] * (scale[o] * sign(w[o,i])) + bias[o]
  with scale[o] = mean_i |w[o,i]|

Sharding: data-parallel over the batch dim (8 batches -> 8 cores); w/bias
replicated. Host passes x and w pre-cast to bf16 (the device matmul is bf16
either way; sign/scale/matmul/bias all stay on device). Per core:

  - x^T built by 4 big XBAR DMA-transposes straight from the bf16 input
    (one per 512-column chunk, full 8 KB source rows), resident in SBUF
  - w binarized on-chip: ACT sign -> bf16 B rows, DVE abs-row-mean -> scale;
    B written to DRAM (gpsimd) and XBAR-transposed back as B^T tiles in
    GROUPS of 2 o-blocks (amortizes the ~5us per-transpose sem latency)
  - ALL DMA_TRANSPOSEs are issued on the sync queue: two concurrent XBAR
    transposes on different HWDGE rings corrupt each other (HW-verified)
  - TensorE: yT[o,m] = B^T.T @ x^T accumulated over k in PSUM; DVE fuses
    psum*scale[o]+bias[o] on eviction; yT writes go out on scalar
  - XBAR transposes exclude ALL other DMA traffic (HWDGE and SWDGE)
    while in flight, so W-prep for the first groups runs before the
    first transpose, and each group's DMA (B^T transpose + W loads +
    B writes + yT writes, ~31us) fits in half its 57.6us compute
  - single pass, block-major: each group of 2 o-blocks computes all 4
    n-chunks back-to-back, so every B^T group is transposed exactly once

Host side only casts/shards inputs and transposes yT shards back into y.
"""

import numpy as np

B_DIM = 8
S_DIM = 2048
IN_F = 4096
OUT_F = 4096
P = 128
N_CORES = 8
N_TILE = 512
G = 2  # o-blocks per B^T transpose group

_BUILT = None


def _build_nc(s_dim=S_DIM, in_f=IN_F, out_f=OUT_F):
    from contextlib import ExitStack

    import concourse.mybir as mybir
    import concourse.tile as tile
    from concourse import bacc
    from concourse.bass import ts

    f32 = mybir.dt.float32
    bf16 = mybir.dt.bfloat16

    NCH = s_dim // N_TILE  # n chunks (moving-dim tiles of 512)
    PO = out_f // P  # o blocks (output-partition tiles of 128)
    KT = in_f // P  # contraction subtiles of 128
    NG = PO // G  # B^T groups
    HALF = in_f // 2

    nc = bacc.Bacc(None, target_bir_lowering=False, debug=False)
    with tile.TileContext(nc) as tc:
        x_d = nc.dram_tensor("x", (s_dim, in_f), bf16, kind="ExternalInput")
        w_d = nc.dram_tensor("w", (out_f, in_f), bf16, kind="ExternalInput")
        b_d = nc.dram_tensor("bias", (out_f,), f32, kind="ExternalInput")
        yT_d = nc.dram_tensor("yT", (out_f, s_dim), f32, kind="ExternalOutput")

        with ExitStack() as ctx:
            dram = ctx.enter_context(tc.tile_pool(name="dram", bufs=1, space="DRAM"))
            # one B-row DRAM tile PER transpose group: a single big tile
            # would make each B^T transpose wait on every previously-emitted
            # B write (tile-granular dependency tracking)
            bwg_d = [
                dram.tile((G * P, in_f), bf16, name=f"bwg{g}")
                for g in range(PO // G)
            ]
            yT3 = yT_d[:, :].rearrange("(po pi) s -> pi po s", pi=P)

            const = ctx.enter_context(tc.tile_pool(name="const", bufs=1))
            # one resident x^T tile PER chunk: a single big tile would make
            # every matmul wait on ALL previously-emitted chunk transposes
            # (tile-granular dependency tracking)
            xTs = [
                const.tile([P, KT, N_TILE], bf16, name=f"xT{c}")
                for c in range(NCH)
            ]
            scale_sb = const.tile([P, PO], f32)
            bias_sb = const.tile([P, PO], f32)
            nc.scalar.dma_start(bias_sb[:], b_d[:].rearrange("(po pi) -> pi po", pi=P))

            wpool = ctx.enter_context(tc.tile_pool(name="wpool", bufs=2))
            bpool = ctx.enter_context(tc.tile_pool(name="bpool", bufs=2))
            scpool = ctx.enter_context(tc.tile_pool(name="scpool", bufs=2))
            btpool = ctx.enter_context(tc.tile_pool(name="btpool", bufs=3))
            opool = ctx.enter_context(tc.tile_pool(name="opool", bufs=4))
            psum = ctx.enter_context(tc.tile_pool(name="psum", bufs=6, space="PSUM"))

            def T_chunk(c):
                # 4 MB XBAR transpose: x rows [512c, 512c+512) -> xTs[c]
                nc.sync.dma_start_transpose(xTs[c][:], x_d[ts(c, N_TILE), :])

            w_tiles = {}

            def load_w(m):
                halves = []
                for h in range(2):
                    w_sb = wpool.tile([P, HALF], bf16, tag="w", name=f"w_{m}_{h}")
                    # SWDGE: keeps W loads off the HWDGE rings, which
                    # serialize against in-flight XBAR transposes
                    nc.gpsimd.dma_start(w_sb[:], w_d[ts(m, P), ts(h, HALF)])
                    halves.append(w_sb)
                w_tiles[m] = halves

            def process_w(m):
                # sign -> bf16 B rows (ACT), |w| row sums -> scale (DVE),
                # B rows -> DRAM (gpsimd SWDGE, off the HWDGE rings)
                sc2 = scpool.tile([P, 2], f32)
                for h in range(2):
                    w_sb = w_tiles[m][h]
                    b_sb = bpool.tile([P, HALF], bf16)
                    nc.scalar.sign(b_sb[:], w_sb[:])
                    nc.vector.tensor_reduce(
                        sc2[:, h : h + 1],
                        w_sb[:],
                        axis=mybir.AxisListType.X,
                        op=mybir.AluOpType.add,
                        apply_absolute_value=True,
                    )
                    nc.gpsimd.dma_start(
                        bwg_d[m // G][ts(m % G, P), ts(h, HALF)], b_sb[:]
                    )
                del w_tiles[m]
                nc.vector.tensor_reduce(
                    scale_sb[:, m : m + 1],
                    sc2[:],
                    axis=mybir.AxisListType.X,
                    op=mybir.AluOpType.add,
                )
                nc.vector.tensor_scalar_mul(
                    scale_sb[:, m : m + 1], scale_sb[:, m : m + 1], 1.0 / in_f
                )

            def load_btg(g):
                # one XBAR transpose covering G consecutive o-blocks:
                # bwg[g] [G*128, in_f] -> [128, KT, G*128]
                bt = btpool.tile([P, KT, G * P], bf16)
                nc.sync.dma_start_transpose(bt[:], bwg_d[g][:, :])
                return bt

            def mm_block(btg, j, m, n):
                ps = psum.tile([P, N_TILE], f32, name="ps")
                for kt in range(KT):
                    nc.tensor.matmul(
                        ps[:],
                        btg[:, kt, ts(j, P)],
                        xTs[n][:, kt, :],
                        start=(kt == 0),
                        stop=(kt == KT - 1),
                    )
                ob = opool.tile([P, N_TILE], f32)
                nc.vector.tensor_scalar(
                    ob[:],
                    ps[:],
                    scale_sb[:, m : m + 1],
                    bias_sb[:, m : m + 1],
                    op0=mybir.AluOpType.mult,
                    op1=mybir.AluOpType.add,
                )
                nc.scalar.dma_start(yT3[:, m, ts(n, N_TILE)], ob[:])

            # W-prep for the first two groups runs BEFORE any transpose
            # (transposes stall all other DMA); then the sync queue runs
            # T0, btg0, T1..T3, and one btg prefetch per group.
            next_proc = 0

            def advance_prep(k=1):
                nonlocal next_proc
                for _ in range(k):
                    if next_proc < PO:
                        load_w(next_proc)
                        process_w(next_proc)
                        next_proc += 1

            advance_prep(3 * G)
            T_chunk(0)
            bt_q = [load_btg(0)]
            if NCH > 1:
                T_chunk(1)
            if NG > 1:
                bt_q.append(load_btg(1))
            for c in range(2, NCH):
                T_chunk(c)

            for g in range(NG):
                btg = bt_q.pop(0)
                advance_prep(G)
                if g + 2 < NG:
                    bt_q.append(load_btg(g + 2))
                for n in range(NCH):
                    for j in range(G):
                        mm_block(btg, j, g * G + j, n)
    nc.finalize()
    return nc


def _get_nc():
    global _BUILT
    if _BUILT is None:
        _BUILT = _build_nc()
    return _BUILT


def kernel(x, weight, bias):
    import ml_dtypes
    from concourse.bass_utils import run_bass_kernel_spmd

    x = np.asarray(x)
    weight = np.asarray(weight)
    bias = np.asarray(bias, dtype=np.float32)
    assert x.shape == (B_DIM, S_DIM, IN_F), x.shape

    x_bf = np.ascontiguousarray(x).astype(ml_dtypes.bfloat16)
    w_bf = np.ascontiguousarray(weight).astype(ml_dtypes.bfloat16)

    nc = _get_nc()
    in_maps = [
        {"x": np.ascontiguousarray(x_bf[b]), "w": w_bf, "bias": bias}
        for b in range(N_CORES)
    ]
    res = run_bass_kernel_spmd(nc, in_maps, core_ids=list(range(N_CORES)))
    out = np.empty((B_DIM, S_DIM, OUT_F), dtype=np.float32)
    for b in range(N_CORES):
        out[b] = res.results[b]["yT"].T
    return out


# revision 19
# speedup vs baseline: 1.0459x; 1.0459x over previous
"""BinaryLinear (sign-binarized weight linear layer) on 8 Trainium2 NeuronCores.

y[b,s,o] = sum_i x[b,s,i] * (scale[o] * sign(w[o,i])) + bias[o]
  with scale[o] = mean_i |w[o,i]|

Sharding: data-parallel over the batch dim (8 batches -> 8 cores); w/bias
replicated. Host passes x and w pre-cast to bf16 (the device matmul is bf16
either way; sign/scale/matmul/bias all stay on device). Per core:

  - x^T built by 4 big XBAR DMA-transposes straight from the bf16 input
    (one per 512-column chunk, full 8 KB source rows), resident in SBUF
  - w binarized on-chip: ACT sign -> bf16 B rows, DVE abs-row-mean -> scale;
    B written to DRAM (gpsimd) and XBAR-transposed back as B^T tiles in
    GROUPS of 2 o-blocks (amortizes the ~5us per-transpose sem latency)
  - ALL DMA_TRANSPOSEs are issued on the sync queue: two concurrent XBAR
    transposes on different HWDGE rings corrupt each other (HW-verified)
  - TensorE: yT[o,m] = B^T.T @ x^T accumulated over k in PSUM; DVE fuses
    psum*scale[o]+bias[o] on eviction; yT writes go out on scalar
  - XBAR transposes exclude ALL other DMA traffic (HWDGE and SWDGE)
    while in flight, so W-prep for the first groups runs before the
    first transpose, and each group's DMA (B^T transpose + W loads +
    B writes + yT writes, ~31us) fits in half its 57.6us compute
  - single pass, block-major: each group of 2 o-blocks computes all 4
    n-chunks back-to-back, so every B^T group is transposed exactly once

Host side only casts/shards inputs and transposes yT shards back into y.
"""

import numpy as np

B_DIM = 8
S_DIM = 2048
IN_F = 4096
OUT_F = 4096
P = 128
N_CORES = 8
N_TILE = 512
G = 2  # o-blocks per B^T transpose group

_BUILT = None


def _build_nc(s_dim=S_DIM, in_f=IN_F, out_f=OUT_F):
    from contextlib import ExitStack

    import concourse.mybir as mybir
    import concourse.tile as tile
    from concourse import bacc
    from concourse.bass import ts

    f32 = mybir.dt.float32
    bf16 = mybir.dt.bfloat16

    NCH = s_dim // N_TILE  # n chunks (moving-dim tiles of 512)
    PO = out_f // P  # o blocks (output-partition tiles of 128)
    KT = in_f // P  # contraction subtiles of 128
    NG = PO // G  # B^T groups
    HALF = in_f // 2

    nc = bacc.Bacc(None, target_bir_lowering=False, debug=False)
    with tile.TileContext(nc) as tc:
        x_d = nc.dram_tensor("x", (s_dim, in_f), bf16, kind="ExternalInput")
        w_d = nc.dram_tensor("w", (out_f, in_f), bf16, kind="ExternalInput")
        b_d = nc.dram_tensor("bias", (out_f,), f32, kind="ExternalInput")
        yT_d = nc.dram_tensor("yT", (out_f, s_dim), f32, kind="ExternalOutput")

        with ExitStack() as ctx:
            dram = ctx.enter_context(tc.tile_pool(name="dram", bufs=1, space="DRAM"))
            # one B-row DRAM tile PER transpose group: a single big tile
            # would make each B^T transpose wait on every previously-emitted
            # B write (tile-granular dependency tracking)
            bwg_d = [
                dram.tile((G * P, in_f), bf16, name=f"bwg{g}")
                for g in range(PO // G)
            ]
            yT3 = yT_d[:, :].rearrange("(po pi) s -> pi po s", pi=P)

            const = ctx.enter_context(tc.tile_pool(name="const", bufs=1))
            # resident x^T in TWO half tiles (2 chunks each): transposed in
            # two 8 MB XBAR ops. Separate tiles so group 0's first rows only
            # wait on half 0 (tile-granular dependency tracking); two ops
            # amortize the per-transpose drain/sem overhead (~5-20us each)
            assert NCH == 4
            xTh = [
                const.tile([P, KT, 2 * N_TILE], bf16, name=f"xTh{h}")
                for h in range(2)
            ]
            scale_sb = const.tile([P, PO], f32)
            bias_sb = const.tile([P, PO], f32)
            nc.scalar.dma_start(bias_sb[:], b_d[:].rearrange("(po pi) -> pi po", pi=P))

            wpool = ctx.enter_context(tc.tile_pool(name="wpool", bufs=2))
            bpool = ctx.enter_context(tc.tile_pool(name="bpool", bufs=2))
            scpool = ctx.enter_context(tc.tile_pool(name="scpool", bufs=2))
            btpool = ctx.enter_context(tc.tile_pool(name="btpool", bufs=3))
            opool = ctx.enter_context(tc.tile_pool(name="opool", bufs=4))
            psum = ctx.enter_context(tc.tile_pool(name="psum", bufs=6, space="PSUM"))

            def T_half(h):
                # 8 MB XBAR transpose: x rows [1024h, 1024h+1024) -> xTh[h]
                nc.sync.dma_start_transpose(
                    xTh[h][:], x_d[ts(h, 2 * N_TILE), :]
                )

            w_tiles = {}

            def load_w(m):
                halves = []
                for h in range(2):
                    w_sb = wpool.tile([P, HALF], bf16, tag="w", name=f"w_{m}_{h}")
                    # SWDGE: keeps W loads off the HWDGE rings, which
                    # serialize against in-flight XBAR transposes
                    nc.gpsimd.dma_start(w_sb[:], w_d[ts(m, P), ts(h, HALF)])
                    halves.append(w_sb)
                w_tiles[m] = halves

            def process_w(m):
                # sign -> bf16 B rows (ACT), |w| row sums -> scale (DVE),
                # B rows -> DRAM (gpsimd SWDGE, off the HWDGE rings)
                sc2 = scpool.tile([P, 2], f32)
                for h in range(2):
                    w_sb = w_tiles[m][h]
                    b_sb = bpool.tile([P, HALF], bf16)
                    nc.scalar.sign(b_sb[:], w_sb[:])
                    nc.vector.tensor_reduce(
                        sc2[:, h : h + 1],
                        w_sb[:],
                        axis=mybir.AxisListType.X,
                        op=mybir.AluOpType.add,
                        apply_absolute_value=True,
                    )
                    nc.gpsimd.dma_start(
                        bwg_d[m // G][ts(m % G, P), ts(h, HALF)], b_sb[:]
                    )
                del w_tiles[m]
                nc.vector.tensor_reduce(
                    scale_sb[:, m : m + 1],
                    sc2[:],
                    axis=mybir.AxisListType.X,
                    op=mybir.AluOpType.add,
                )
                nc.vector.tensor_scalar_mul(
                    scale_sb[:, m : m + 1], scale_sb[:, m : m + 1], 1.0 / in_f
                )

            def load_btg(g):
                # one XBAR transpose covering G consecutive o-blocks:
                # bwg[g] [G*128, in_f] -> [128, KT, G*128]
                bt = btpool.tile([P, KT, G * P], bf16)
                nc.sync.dma_start_transpose(bt[:], bwg_d[g][:, :])
                return bt

            def mm_block(btg, j, m, n):
                ps = psum.tile([P, N_TILE], f32, name="ps")
                for kt in range(KT):
                    nc.tensor.matmul(
                        ps[:],
                        btg[:, kt, ts(j, P)],
                        xTh[n // 2][:, kt, ts(n % 2, N_TILE)],
                        start=(kt == 0),
                        stop=(kt == KT - 1),
                    )
                ob = opool.tile([P, N_TILE], f32)
                nc.vector.tensor_scalar(
                    ob[:],
                    ps[:],
                    scale_sb[:, m : m + 1],
                    bias_sb[:, m : m + 1],
                    op0=mybir.AluOpType.mult,
                    op1=mybir.AluOpType.add,
                )
                nc.scalar.dma_start(yT3[:, m, ts(n, N_TILE)], ob[:])

            # W-prep for the first two groups runs BEFORE any transpose
            # (transposes stall all other DMA); then the sync queue runs
            # T0, btg0, T1..T3, and one btg prefetch per group.
            next_proc = 0

            def advance_prep(k=1):
                nonlocal next_proc
                for _ in range(k):
                    if next_proc < PO:
                        load_w(next_proc)
                        process_w(next_proc)
                        next_proc += 1

            advance_prep(G)
            T_half(0)
            bt_q = [load_btg(0)]
            advance_prep(G)
            T_half(1)
            if NG > 1:
                bt_q.append(load_btg(1))
            advance_prep(G)

            for g in range(NG):
                btg = bt_q.pop(0)
                advance_prep(G)
                if g + 2 < NG:
                    bt_q.append(load_btg(g + 2))
                for n in range(NCH):
                    for j in range(G):
                        mm_block(btg, j, g * G + j, n)
    nc.finalize()
    return nc


def _get_nc():
    global _BUILT
    if _BUILT is None:
        _BUILT = _build_nc()
    return _BUILT


def kernel(x, weight, bias):
    import ml_dtypes
    from concourse.bass_utils import run_bass_kernel_spmd

    x = np.asarray(x)
    weight = np.asarray(weight)
    bias = np.asarray(bias, dtype=np.float32)
    assert x.shape == (B_DIM, S_DIM, IN_F), x.shape

    x_bf = np.ascontiguousarray(x).astype(ml_dtypes.bfloat16)
    w_bf = np.ascontiguousarray(weight).astype(ml_dtypes.bfloat16)

    nc = _get_nc()
    in_maps = [
        {"x": np.ascontiguousarray(x_bf[b]), "w": w_bf, "bias": bias}
        for b in range(N_CORES)
    ]
    res = run_bass_kernel_spmd(nc, in_maps, core_ids=list(range(N_CORES)))
    out = np.empty((B_DIM, S_DIM, OUT_F), dtype=np.float32)
    for b in range(N_CORES):
        out[b] = res.results[b]["yT"].T
    return out
